# revision 17
# baseline (speedup 1.0000x reference)
"""Trainium2 Bass kernel for nn_Convs4x44 (dense_cnn, memory-bound).

Pipeline per sample (64 input floats -> 4 output floats):
  conv1 2x2/s2 on 8x8 -> relu(x-0.2) -> conv2 2x2/s2 on 4x4 -> relu(x-2)
  -> 4->3 linear + relu -> 3->2 linear -> softmax(2)

Strategy: pure data parallel over 8 cores. Sample-major SBUF layout
[128 partitions, C samples/partition * 64 feats] so both elementwise engines
run with all 128 lanes busy and the HBM loads are big contiguous blocks.
Convs/MLP are fused scalar_tensor_tensor accumulation chains
(out = (x_tap * k) + acc) on DVE, with the leading product of each chain and
all ReLU/sigmoid on ACT; softmax(2) is computed exactly as
sigmoid(+/-(raw0-raw1)). GPSIMD is deliberately idle: its elementwise ops
measured 3-20x slower and its SBUF port contention slowed concurrent DVE ops
~3x.
"""

import numpy as np

import concourse.bass as bass
import concourse.tile as tile
from concourse import mybir
from concourse.bass_utils import run_bass_kernel_spmd


def _split_multiwaits(nc):
    """This container's walrus build supports only ONE sync-wait command per
    instruction ("Too many sync wait commands" otherwise), while Tile freely
    emits multi-wait instructions. Split every instruction with N>1 waits
    into (N-1) same-engine NoOps carrying one wait each, inserted before it
    in the basic block; per-engine execution order is block order filtered
    by engine, so semantics are unchanged."""
    for func in nc.m.functions:
        for blk in func.blocks:
            insts = blk.instructions
            out = []
            changed = False
            for ins in insts:
                si = ins.sync_info
                if si is not None and len(si.on_wait) > 1:
                    waits = list(si.on_wait)
                    for k, w in enumerate(waits[:-1]):
                        nop = mybir.InstNoOp(
                            name=f"{ins.name}-wsplit-{k}", ins=[], outs=[])
                        nop.engine = ins.engine
                        nop.sync_info = mybir.SyncInfo(on_wait=[w], on_update=[])
                        out.append(nop)
                    ins.sync_info = mybir.SyncInfo(
                        on_wait=[waits[-1]], on_update=list(si.on_update))
                    changed = True
                out.append(ins)
            if changed:
                insts[:] = out


N_CORES = 8
B = 1048576
BC = B // N_CORES          # samples per core
P = 128                    # SBUF partitions
# Conv-tile samples-per-partition. Ramped small-first so the first
# compute tile's x load (the serial head of the pipeline) is short; the
# x-load DMA chain on the SP HWDGE ring is the end-to-end bottleneck, so
# later tiles grow to amortize per-op overhead. 64-tail keeps the
# last-compute + store tail short. Sum must be BC / P = 1024.
TILE_CS = [96, 160, 160, 160, 160, 160, 96, 32]
# MLP stages run once per GROUP of conv tiles. Singleton groups keep the
# group tiles small (SBUF) and the tail MLP latency short; the DVE
# per-op overhead this costs is hidden in the DMA-bound steady state.
GROUPS = [(i,) for i in range(len(TILE_CS))]
assert sum(TILE_CS) * P == BC

F32 = mybir.dt.float32
ALU = mybir.AluOpType
AF = mybir.ActivationFunctionType

# columns in the broadcast-constant tile
K1 = 0            # conv1_w taps  [k00,k01,k10,k11]
K2 = 4            # conv2_w taps
W1C = 8           # W1[j,i] -> 8 + 4j + i
B1C = 20          # b1[j]
W2C = 23          # W2[j,i] -> 23 + 3j + i
B2C = 29          # b2[j]
SH1 = 31          # -0.2 (conv1 relu shift)
SH2 = 32          # -2.0 (f relu shift)
NW = 33


def _build(repeat=1):
    """repeat>1 unrolls the whole pipeline R times inside one NEFF —
    used only by bench.py to measure steady-state per-pass time with a
    strong signal (device busy R*~100us >> dispatch noise)."""
    nc = bass.Bass("TRN2", target_bir_lowering=False, debug=False,
                   num_devices=N_CORES)
    x = nc.dram_tensor("x", [BC, 64], F32, kind="ExternalInput")
    wconst = nc.dram_tensor("wconst", [P, NW], F32, kind="ExternalInput")
    out = nc.dram_tensor("out", [BC, 4], F32, kind="ExternalOutput")

    CMAX = max(TILE_CS)
    GMAX = max(sum(TILE_CS[i] for i in g) for g in GROUPS)

    with tile.TileContext(nc) as tc:
        with (
            tc.tile_pool(name="consts", bufs=1) as cpool,
            tc.tile_pool(name="x", bufs=3) as xpool,
            tc.tile_pool(name="mid", bufs=2) as mpool,
            tc.tile_pool(name="small", bufs=2) as spool,
            tc.tile_pool(name="grp", bufs=2) as gpool,
            tc.tile_pool(name="out", bufs=2) as opool,
        ):
            ws = cpool.tile([P, NW], F32)
            nc.sync.dma_start(ws[:], wconst.ap()[:])

            def sc(col):
                return ws[:, col:col + 1]

            # ACT table warmup during the first x load: touch every
            # activation set used below so the ~2.7us PSEUDO_LOAD fires
            # at t~1us instead of ahead of the first compute tile.
            scratch = cpool.tile([P, 4], F32)
            nc.scalar.activation(scratch[:, 0:1], ws[:, SH1:SH1 + 1],
                                 AF.Relu, bias=0.0, scale=1.0)
            nc.scalar.activation(scratch[:, 1:2], ws[:, SH1:SH1 + 1],
                                 AF.Identity, bias=0.0, scale=1.0)
            nc.scalar.activation(scratch[:, 2:3], ws[:, SH1:SH1 + 1],
                                 AF.Sigmoid, bias=0.0, scale=1.0)

            def conv_tile(C, row0):
                """Load + conv1 + relu + conv2 + shifted-relu; returns the
                per-tile f tile ([P, C*4], f_i per sample)."""
                x_view = x.ap()[row0:row0 + P * C, :].rearrange(
                    "(p c) f -> p (c f)", p=P, c=C)
                xt = xpool.tile([P, CMAX * 64], F32, tag="xt")
                nc.sync.dma_start(xt[:, :C * 64], x_view)

                # conv1: t1[c,oh,ow] = sum_t k[ti,tj]*x[c,2oh+ti,2ow+tj]
                xv = xt[:, :C * 64].rearrange(
                    "p (c oh ti ow tj) -> p c oh ti ow tj", oh=4, ti=2,
                    ow=4, tj=2)
                t1 = mpool.tile([P, CMAX * 16], F32, tag="t1")
                t1v = t1[:, :C * 16].rearrange("p (c oh ow) -> p c oh ow",
                                               oh=4, ow=4)
                nc.scalar.activation(t1v, xv[:, :, :, 0, :, 0], AF.Copy,
                                     bias=0.0, scale=sc(K1 + 0))
                nc.vector.scalar_tensor_tensor(
                    t1v, xv[:, :, :, 0, :, 1], sc(K1 + 1), t1v,
                    ALU.mult, ALU.add)
                nc.vector.scalar_tensor_tensor(
                    t1v, xv[:, :, :, 1, :, 0], sc(K1 + 2), t1v,
                    ALU.mult, ALU.add)
                nc.vector.scalar_tensor_tensor(
                    t1v, xv[:, :, :, 1, :, 1], sc(K1 + 3), t1v,
                    ALU.mult, ALU.add)

                # relu(x1 - 0.2)
                x1r = mpool.tile([P, CMAX * 16], F32, tag="x1r")
                nc.scalar.activation(x1r[:, :C * 16], t1[:, :C * 16],
                                     AF.Relu, bias=sc(SH1), scale=1.0)

                # conv2 on the 4x4 maps
                x1v = x1r[:, :C * 16].rearrange(
                    "p (c oh ti ow tj) -> p c oh ti ow tj", oh=2, ti=2,
                    ow=2, tj=2)
                t2 = spool.tile([P, CMAX * 4], F32, tag="t2")
                t2v = t2[:, :C * 4].rearrange("p (c oh ow) -> p c oh ow",
                                              oh=2, ow=2)
                # conv2 accumulate chain runs on GPSIMD (otherwise idle)
                # to take 12 of the 74 DVE elem-ops/sample off the
                # bottleneck engine. Walrus cannot lower STT on Pool, so
                # each tap is a tensor_scalar mult into a temp plus a
                # tensor_tensor add.
                nc.scalar.activation(t2v, x1v[:, :, :, 0, :, 0], AF.Copy,
                                     bias=0.0, scale=sc(K2 + 0))
                t2b = spool.tile([P, CMAX * 4], F32, tag="t2b")
                t2bv = t2b[:, :C * 4].rearrange("p (c oh ow) -> p c oh ow",
                                                oh=2, ow=2)
                for ti, tj, col in ((0, 1, K2 + 1), (1, 0, K2 + 2),
                                    (1, 1, K2 + 3)):
                    nc.gpsimd.tensor_scalar_mul(t2bv, x1v[:, :, :, ti, :, tj],
                                                sc(col))
                    nc.gpsimd.tensor_tensor(t2v, t2v, t2bv, ALU.add)

                # f = relu(x2 - 2)
                f = gpool.tile([P, GMAX * 4], F32, tag="f")
                nc.scalar.activation(f[:, :C * 4], t2[:, :C * 4], AF.Relu,
                                     bias=sc(SH2), scale=1.0)
                return f

            def mlp_tile(f, G, row0):
                """4->3 relu MLP, 3->2 linear, softmax(2), store."""
                fv = f[:, :G * 4].rearrange("p (c i) -> p c i", i=4)
                # h_j = relu(sum_i W1[j,i] f_i + b1_j), stored j-major
                h = gpool.tile([P, GMAX * 3], F32, tag="h")
                for j in range(3):
                    hj = h[:, j * G:(j + 1) * G]
                    nc.scalar.activation(hj, fv[:, :, 0], AF.Identity,
                                         bias=sc(B1C + j),
                                         scale=sc(W1C + 4 * j))
                    for i in range(1, 4):
                        nc.vector.scalar_tensor_tensor(
                            hj, fv[:, :, i], sc(W1C + 4 * j + i), hj,
                            ALU.mult, ALU.add)
                hr = gpool.tile([P, GMAX * 3], F32, tag="hr")
                nc.scalar.activation(hr[:, :G * 3], h[:, :G * 3], AF.Relu,
                                     bias=0.0, scale=1.0)
                hrv = hr[:, :G * 3].rearrange("p (j c) -> p j c", j=3)

                # out layout per sample: [cls0, cls1, raw0, raw1]
                ot = opool.tile([P, GMAX * 4], F32, tag="ot")
                ov = ot[:, :G * 4].rearrange("p (c four) -> p c four", four=4)
                for j in range(2):
                    rj = ov[:, :, 2 + j]
                    nc.scalar.activation(rj, hrv[:, 0, :], AF.Identity,
                                         bias=sc(B2C + j),
                                         scale=sc(W2C + 3 * j))
                    for i in range(1, 3):
                        nc.vector.scalar_tensor_tensor(
                            rj, hrv[:, i, :], sc(W2C + 3 * j + i), rj,
                            ALU.mult, ALU.add)

                # softmax over 2 classes: cls0 = sigmoid(raw0-raw1)
                d = gpool.tile([P, GMAX], F32, tag="d")
                nc.vector.tensor_sub(d[:, :G], ov[:, :, 2], ov[:, :, 3])
                nc.scalar.activation(ov[:, :, 0], d[:, :G], AF.Sigmoid,
                                     bias=0.0, scale=1.0)
                nc.scalar.activation(ov[:, :, 1], d[:, :G], AF.Sigmoid,
                                     bias=0.0, scale=-1.0)

                # Store on the SP ring: at ~716 GB/s/core the load chain
                # has plenty of slack, and keeping stores off the ACT
                # queue frees the ACT engine (a near-co-bottleneck).
                out_view = out.ap()[row0:row0 + P * G, :].rearrange(
                    "(p c) four -> p (c four)", p=P, c=G)
                nc.sync.dma_start(out_view, ot[:, :G * 4])

            # Software-pipelined emission: tile i's conv ops are issued
            # before tile i-1's MLP so the next tile's conv leads never
            # queue behind a full MLP chain on the ACT engine.
            for _rep in range(repeat):
                s0 = 0
                pending = None          # (f, C, row0) awaiting MLP
                for C in TILE_CS:
                    fcur = conv_tile(C, s0)
                    if pending is not None:
                        mlp_tile(*pending)
                    pending = (fcur, C, s0)
                    s0 += P * C
                mlp_tile(*pending)

    _split_multiwaits(nc)
    return nc


_NC = {}
LAST_RESULT = None  # BassKernelResults of the most recent kernel() call


def _get_nc(repeat=1):
    if repeat not in _NC:
        _NC[repeat] = _build(repeat)
    return _NC[repeat]


def prep_in_maps(x, conv1_w, conv2_w, W1, b1, W2, b2):
    x = np.ascontiguousarray(np.asarray(x, dtype=np.float32)).reshape(B, 64)
    row = np.concatenate([
        np.asarray(conv1_w, dtype=np.float32).reshape(4),
        np.asarray(conv2_w, dtype=np.float32).reshape(4),
        np.asarray(W1, dtype=np.float32).reshape(12),
        np.asarray(b1, dtype=np.float32).reshape(3),
        np.asarray(W2, dtype=np.float32).reshape(6),
        np.asarray(b2, dtype=np.float32).reshape(2),
        np.array([-0.2, -2.0], dtype=np.float32),
    ])
    wconst = np.ascontiguousarray(np.tile(row[None, :], (P, 1)))
    return [
        {"x": np.ascontiguousarray(x[i * BC:(i + 1) * BC]), "wconst": wconst}
        for i in range(N_CORES)
    ]


def kernel(x, conv1_w, conv2_w, W1, b1, W2, b2):
    in_maps = prep_in_maps(x, conv1_w, conv2_w, W1, b1, W2, b2)
    nc = _get_nc()
    res = run_bass_kernel_spmd(nc, in_maps, core_ids=list(range(N_CORES)))
    global LAST_RESULT
    LAST_RESULT = res
    out = np.concatenate([res.results[i]["out"] for i in range(N_CORES)], axis=0)
    classification = np.ascontiguousarray(out[:, 0:2])
    raw = np.ascontiguousarray(out[:, 2:4])
    return classification, raw



# revision 19
# speedup vs baseline: 1.9216x; 1.9216x over previous
"""Trainium2 Bass kernel for nn_Convs4x44 (dense_cnn, memory-bound).

Pipeline per sample (64 input floats -> 4 output floats):
  conv1 2x2/s2 on 8x8 -> relu(x-0.2) -> conv2 2x2/s2 on 4x4 -> relu(x-2)
  -> 4->3 linear + relu -> 3->2 linear -> softmax(2)

Strategy: pure data parallel over 8 cores. Sample-major SBUF layout
[128 partitions, C samples/partition * 64 feats] so both elementwise engines
run with all 128 lanes busy and the HBM loads are big contiguous blocks.
Convs/MLP are fused scalar_tensor_tensor accumulation chains
(out = (x_tap * k) + acc) on DVE, with the leading product of each chain and
all ReLU/sigmoid on ACT; softmax(2) is computed exactly as
sigmoid(+/-(raw0-raw1)).

Measured facts shaping the schedule (see bench.py for the methodology):
- Per-core HBM bandwidth here is ~716 GB/s (devices on separate stack
  pairs), so DMA has ~2x slack and the kernel is DVE-bound: 74 DVE
  elem-ops/sample at 1 elem/cycle fp32 == the measured ~70us/pass.
- GPSIMD is deliberately idle: walrus cannot lower STT on Pool, and a
  tensor_scalar+tensor_tensor conv2 offload regressed per-pass 70us ->
  240us (SBUF port contention with DVE).
- bf16 is forbidden: `raw` crosses zero (min |exp| ~6e-5), so the
  max-rel-err metric amplifies any added absolute error; fp32 keeps the
  kernel at rel err 2.9e-3 vs the 2e-2 gate.
- Emission is software-pipelined (tile i's convs before tile i-1's MLP)
  and tiles ramp small-large-small so the after-last-load tail (a
  latency chain of small ops + cross-engine hops) stays short.
"""

import numpy as np

import concourse.bass as bass
import concourse.tile as tile
from concourse import mybir
from concourse.bass_utils import run_bass_kernel_spmd


def _split_multiwaits(nc):
    """This container's walrus build supports only ONE sync-wait command per
    instruction ("Too many sync wait commands" otherwise), while Tile freely
    emits multi-wait instructions. Split every instruction with N>1 waits
    into (N-1) same-engine NoOps carrying one wait each, inserted before it
    in the basic block; per-engine execution order is block order filtered
    by engine, so semantics are unchanged."""
    for func in nc.m.functions:
        for blk in func.blocks:
            insts = blk.instructions
            out = []
            changed = False
            for ins in insts:
                si = ins.sync_info
                if si is not None and len(si.on_wait) > 1:
                    waits = list(si.on_wait)
                    for k, w in enumerate(waits[:-1]):
                        nop = mybir.InstNoOp(
                            name=f"{ins.name}-wsplit-{k}", ins=[], outs=[])
                        nop.engine = ins.engine
                        nop.sync_info = mybir.SyncInfo(on_wait=[w], on_update=[])
                        out.append(nop)
                    ins.sync_info = mybir.SyncInfo(
                        on_wait=[waits[-1]], on_update=list(si.on_update))
                    changed = True
                out.append(ins)
            if changed:
                insts[:] = out


N_CORES = 8
B = 1048576
BC = B // N_CORES          # samples per core
P = 128                    # SBUF partitions
# Conv-tile samples-per-partition. Ramped small-first so the first
# compute tile's x load (the serial head of the pipeline) is short; the
# x-load DMA chain on the SP HWDGE ring is the end-to-end bottleneck, so
# later tiles grow to amortize per-op overhead. 64-tail keeps the
# last-compute + store tail short. Sum must be BC / P = 1024.
TILE_CS = [96, 160, 160, 160, 160, 160, 96, 32]
# MLP stages run once per GROUP of conv tiles. Singleton groups keep the
# group tiles small (SBUF) and the tail MLP latency short; the DVE
# per-op overhead this costs is hidden in the DMA-bound steady state.
GROUPS = [(i,) for i in range(len(TILE_CS))]
assert sum(TILE_CS) * P == BC

F32 = mybir.dt.float32
ALU = mybir.AluOpType
AF = mybir.ActivationFunctionType

# columns in the broadcast-constant tile
K1 = 0            # conv1_w taps  [k00,k01,k10,k11]
K2 = 4            # conv2_w taps
W1C = 8           # W1[j,i] -> 8 + 4j + i
B1C = 20          # b1[j]
W2C = 23          # W2[j,i] -> 23 + 3j + i
B2C = 29          # b2[j]
SH1 = 31          # -0.2 (conv1 relu shift)
SH2 = 32          # -2.0 (f relu shift)
NW = 33


def _build(repeat=1):
    """repeat>1 unrolls the whole pipeline R times inside one NEFF —
    used only by bench.py to measure steady-state per-pass time with a
    strong signal (device busy R*~100us >> dispatch noise)."""
    nc = bass.Bass("TRN2", target_bir_lowering=False, debug=False,
                   num_devices=N_CORES)
    x = nc.dram_tensor("x", [BC, 64], F32, kind="ExternalInput")
    wconst = nc.dram_tensor("wconst", [P, NW], F32, kind="ExternalInput")
    out = nc.dram_tensor("out", [BC, 4], F32, kind="ExternalOutput")

    CMAX = max(TILE_CS)
    GMAX = max(sum(TILE_CS[i] for i in g) for g in GROUPS)

    with tile.TileContext(nc) as tc:
        with (
            tc.tile_pool(name="consts", bufs=1) as cpool,
            tc.tile_pool(name="x", bufs=3) as xpool,
            tc.tile_pool(name="mid", bufs=2) as mpool,
            tc.tile_pool(name="small", bufs=2) as spool,
            tc.tile_pool(name="grp", bufs=2) as gpool,
            tc.tile_pool(name="out", bufs=2) as opool,
        ):
            ws = cpool.tile([P, NW], F32)
            nc.sync.dma_start(ws[:], wconst.ap()[:])

            def sc(col):
                return ws[:, col:col + 1]

            # ACT table warmup during the first x load: touch every
            # activation set used below so the ~2.7us PSEUDO_LOAD fires
            # at t~1us instead of ahead of the first compute tile.
            scratch = cpool.tile([P, 4], F32)
            nc.scalar.activation(scratch[:, 0:1], ws[:, SH1:SH1 + 1],
                                 AF.Relu, bias=0.0, scale=1.0)
            nc.scalar.activation(scratch[:, 1:2], ws[:, SH1:SH1 + 1],
                                 AF.Identity, bias=0.0, scale=1.0)
            nc.scalar.activation(scratch[:, 2:3], ws[:, SH1:SH1 + 1],
                                 AF.Sigmoid, bias=0.0, scale=1.0)

            def conv_tile(C, row0):
                """Load + conv1 + relu + conv2 + shifted-relu; returns the
                per-tile f tile ([P, C*4], f_i per sample)."""
                x_view = x.ap()[row0:row0 + P * C, :].rearrange(
                    "(p c) f -> p (c f)", p=P, c=C)
                xt = xpool.tile([P, CMAX * 64], F32, tag="xt")
                nc.sync.dma_start(xt[:, :C * 64], x_view)

                # conv1: t1[c,oh,ow] = sum_t k[ti,tj]*x[c,2oh+ti,2ow+tj]
                xv = xt[:, :C * 64].rearrange(
                    "p (c oh ti ow tj) -> p c oh ti ow tj", oh=4, ti=2,
                    ow=4, tj=2)
                t1 = mpool.tile([P, CMAX * 16], F32, tag="t1")
                t1v = t1[:, :C * 16].rearrange("p (c oh ow) -> p c oh ow",
                                               oh=4, ow=4)
                nc.scalar.activation(t1v, xv[:, :, :, 0, :, 0], AF.Copy,
                                     bias=0.0, scale=sc(K1 + 0))
                nc.vector.scalar_tensor_tensor(
                    t1v, xv[:, :, :, 0, :, 1], sc(K1 + 1), t1v,
                    ALU.mult, ALU.add)
                nc.vector.scalar_tensor_tensor(
                    t1v, xv[:, :, :, 1, :, 0], sc(K1 + 2), t1v,
                    ALU.mult, ALU.add)
                nc.vector.scalar_tensor_tensor(
                    t1v, xv[:, :, :, 1, :, 1], sc(K1 + 3), t1v,
                    ALU.mult, ALU.add)

                # relu(x1 - 0.2)
                x1r = mpool.tile([P, CMAX * 16], F32, tag="x1r")
                nc.scalar.activation(x1r[:, :C * 16], t1[:, :C * 16],
                                     AF.Relu, bias=sc(SH1), scale=1.0)

                # conv2 on the 4x4 maps
                x1v = x1r[:, :C * 16].rearrange(
                    "p (c oh ti ow tj) -> p c oh ti ow tj", oh=2, ti=2,
                    ow=2, tj=2)
                t2 = spool.tile([P, CMAX * 4], F32, tag="t2")
                t2v = t2[:, :C * 4].rearrange("p (c oh ow) -> p c oh ow",
                                              oh=2, ow=2)
                # conv2 accumulate stays on DVE: GPSIMD elementwise was
                # measured 3x+ slower with SBUF port contention (a
                # ts_mul+tt_add variant regressed per-pass 70us -> 240us),
                # and walrus cannot lower STT on Pool at all.
                nc.scalar.activation(t2v, x1v[:, :, :, 0, :, 0], AF.Copy,
                                     bias=0.0, scale=sc(K2 + 0))
                nc.vector.scalar_tensor_tensor(
                    t2v, x1v[:, :, :, 0, :, 1], sc(K2 + 1), t2v,
                    ALU.mult, ALU.add)
                nc.vector.scalar_tensor_tensor(
                    t2v, x1v[:, :, :, 1, :, 0], sc(K2 + 2), t2v,
                    ALU.mult, ALU.add)
                nc.vector.scalar_tensor_tensor(
                    t2v, x1v[:, :, :, 1, :, 1], sc(K2 + 3), t2v,
                    ALU.mult, ALU.add)

                # f = relu(x2 - 2)
                f = gpool.tile([P, GMAX * 4], F32, tag="f")
                nc.scalar.activation(f[:, :C * 4], t2[:, :C * 4], AF.Relu,
                                     bias=sc(SH2), scale=1.0)
                return f

            def mlp_tile(f, G, row0):
                """4->3 relu MLP, 3->2 linear, softmax(2), store."""
                fv = f[:, :G * 4].rearrange("p (c i) -> p c i", i=4)
                # h_j = relu(sum_i W1[j,i] f_i + b1_j), stored j-major
                h = gpool.tile([P, GMAX * 3], F32, tag="h")
                for j in range(3):
                    hj = h[:, j * G:(j + 1) * G]
                    nc.scalar.activation(hj, fv[:, :, 0], AF.Identity,
                                         bias=sc(B1C + j),
                                         scale=sc(W1C + 4 * j))
                    for i in range(1, 4):
                        nc.vector.scalar_tensor_tensor(
                            hj, fv[:, :, i], sc(W1C + 4 * j + i), hj,
                            ALU.mult, ALU.add)
                hr = gpool.tile([P, GMAX * 3], F32, tag="hr")
                nc.scalar.activation(hr[:, :G * 3], h[:, :G * 3], AF.Relu,
                                     bias=0.0, scale=1.0)
                hrv = hr[:, :G * 3].rearrange("p (j c) -> p j c", j=3)

                # out layout per sample: [cls0, cls1, raw0, raw1]
                ot = opool.tile([P, GMAX * 4], F32, tag="ot")
                ov = ot[:, :G * 4].rearrange("p (c four) -> p c four", four=4)
                for j in range(2):
                    rj = ov[:, :, 2 + j]
                    nc.scalar.activation(rj, hrv[:, 0, :], AF.Identity,
                                         bias=sc(B2C + j),
                                         scale=sc(W2C + 3 * j))
                    for i in range(1, 3):
                        nc.vector.scalar_tensor_tensor(
                            rj, hrv[:, i, :], sc(W2C + 3 * j + i), rj,
                            ALU.mult, ALU.add)

                # softmax over 2 classes: cls0 = sigmoid(raw0-raw1)
                d = gpool.tile([P, GMAX], F32, tag="d")
                nc.vector.tensor_sub(d[:, :G], ov[:, :, 2], ov[:, :, 3])
                nc.scalar.activation(ov[:, :, 0], d[:, :G], AF.Sigmoid,
                                     bias=0.0, scale=1.0)
                nc.scalar.activation(ov[:, :, 1], d[:, :G], AF.Sigmoid,
                                     bias=0.0, scale=-1.0)

                # Store on the SP ring: at ~716 GB/s/core the load chain
                # has plenty of slack, and keeping stores off the ACT
                # queue frees the ACT engine (a near-co-bottleneck).
                out_view = out.ap()[row0:row0 + P * G, :].rearrange(
                    "(p c) four -> p (c four)", p=P, c=G)
                nc.sync.dma_start(out_view, ot[:, :G * 4])

            # Software-pipelined emission: tile i's conv ops are issued
            # before tile i-1's MLP so the next tile's conv leads never
            # queue behind a full MLP chain on the ACT engine.
            for _rep in range(repeat):
                s0 = 0
                pending = None          # (f, C, row0) awaiting MLP
                for C in TILE_CS:
                    fcur = conv_tile(C, s0)
                    if pending is not None:
                        mlp_tile(*pending)
                    pending = (fcur, C, s0)
                    s0 += P * C
                mlp_tile(*pending)

    _split_multiwaits(nc)
    return nc


_NC = {}
LAST_RESULT = None  # BassKernelResults of the most recent kernel() call


def _get_nc(repeat=1):
    if repeat not in _NC:
        _NC[repeat] = _build(repeat)
    return _NC[repeat]


def prep_in_maps(x, conv1_w, conv2_w, W1, b1, W2, b2):
    x = np.ascontiguousarray(np.asarray(x, dtype=np.float32)).reshape(B, 64)
    row = np.concatenate([
        np.asarray(conv1_w, dtype=np.float32).reshape(4),
        np.asarray(conv2_w, dtype=np.float32).reshape(4),
        np.asarray(W1, dtype=np.float32).reshape(12),
        np.asarray(b1, dtype=np.float32).reshape(3),
        np.asarray(W2, dtype=np.float32).reshape(6),
        np.asarray(b2, dtype=np.float32).reshape(2),
        np.array([-0.2, -2.0], dtype=np.float32),
    ])
    wconst = np.ascontiguousarray(np.tile(row[None, :], (P, 1)))
    return [
        {"x": np.ascontiguousarray(x[i * BC:(i + 1) * BC]), "wconst": wconst}
        for i in range(N_CORES)
    ]


def kernel(x, conv1_w, conv2_w, W1, b1, W2, b2):
    in_maps = prep_in_maps(x, conv1_w, conv2_w, W1, b1, W2, b2)
    nc = _get_nc()
    res = run_bass_kernel_spmd(nc, in_maps, core_ids=list(range(N_CORES)))
    global LAST_RESULT
    LAST_RESULT = res
    out = np.concatenate([res.results[i]["out"] for i in range(N_CORES)], axis=0)
    classification = np.ascontiguousarray(out[:, 0:2])
    raw = np.ascontiguousarray(out[:, 2:4])
    return classification, raw



# revision 24
# speedup vs baseline: 3.0075x; 1.5651x over previous
"""Trainium2 Bass kernel for nn_Convs4x44 (dense_cnn, memory-bound).

Pipeline per sample (64 input floats -> 4 output floats):
  conv1 2x2/s2 on 8x8 -> relu(x-0.2) -> conv2 2x2/s2 on 4x4 -> relu(x-2)
  -> 4->3 linear + relu -> 3->2 linear -> softmax(2)

Strategy: pure data parallel over 8 cores. Sample-major SBUF layout
[128 partitions, C samples/partition * 64 feats] so both elementwise engines
run with all 128 lanes busy and the HBM loads are big contiguous blocks.
Convs/MLP are fused scalar_tensor_tensor accumulation chains
(out = (x_tap * k) + acc) on DVE, with the leading product of each chain and
all ReLU/sigmoid on ACT; softmax(2) is computed exactly as
sigmoid(+/-(raw0-raw1)).

Measured facts shaping the schedule (see bench.py for the methodology):
- Per-core HBM bandwidth here is ~716 GB/s (devices on separate stack
  pairs), so DMA has ~2x slack and the kernel is DVE-bound: 74 DVE
  elem-ops/sample at 1 elem/cycle fp32 == the measured ~70us/pass.
- GPSIMD is deliberately idle: walrus cannot lower STT on Pool, and a
  tensor_scalar+tensor_tensor conv2 offload regressed per-pass 70us ->
  240us (SBUF port contention with DVE).
- bf16 is forbidden: `raw` crosses zero (min |exp| ~6e-5), so the
  max-rel-err metric amplifies any added absolute error; fp32 keeps the
  kernel at rel err 2.9e-3 vs the 2e-2 gate.
- Emission is software-pipelined (tile i's convs before tile i-1's MLP)
  and tiles ramp small-large-small so the after-last-load tail (a
  latency chain of small ops + cross-engine hops) stays short.
"""

import numpy as np

import concourse.bass as bass
import concourse.tile as tile
from concourse import mybir
from concourse.bass_utils import run_bass_kernel_spmd


def _split_multiwaits(nc):
    """This container's walrus build supports only ONE sync-wait command per
    instruction ("Too many sync wait commands" otherwise), while Tile freely
    emits multi-wait instructions. Split every instruction with N>1 waits
    into (N-1) same-engine NoOps carrying one wait each, inserted before it
    in the basic block; per-engine execution order is block order filtered
    by engine, so semantics are unchanged."""
    for func in nc.m.functions:
        for blk in func.blocks:
            insts = blk.instructions
            out = []
            changed = False
            for ins in insts:
                si = ins.sync_info
                if si is not None and len(si.on_wait) > 1:
                    waits = list(si.on_wait)
                    for k, w in enumerate(waits[:-1]):
                        nop = mybir.InstNoOp(
                            name=f"{ins.name}-wsplit-{k}", ins=[], outs=[])
                        nop.engine = ins.engine
                        nop.sync_info = mybir.SyncInfo(on_wait=[w], on_update=[])
                        out.append(nop)
                    ins.sync_info = mybir.SyncInfo(
                        on_wait=[waits[-1]], on_update=list(si.on_update))
                    changed = True
                out.append(ins)
            if changed:
                insts[:] = out


N_CORES = 8
B = 1048576
BC = B // N_CORES          # samples per core
P = 128                    # SBUF partitions
# Conv-tile samples-per-partition. Ramped small-first so the first
# compute tile's x load (the serial head of the pipeline) is short; the
# x-load DMA chain on the SP HWDGE ring is the end-to-end bottleneck, so
# later tiles grow to amortize per-op overhead. 64-tail keeps the
# last-compute + store tail short. Sum must be BC / P = 1024.
TILE_CS = [96, 192, 192, 192, 192, 128, 32]
# MLP stages run once per GROUP of conv tiles: pairs mid-stream so the
# 14 small MLP ops amortize their ~100-cycle DVE per-op overhead over
# 2x the elements, singleton at the tail so the last MLP latency chain
# stays short. DVE op count/pass: 42 conv + 56 MLP = 98 (vs 152 with
# singleton groups) — the kernel is DVE-bound, so op overhead is time.
GROUPS = [(0, 1), (2, 3), (4, 5), (6,)]
assert sum(TILE_CS) * P == BC

F32 = mybir.dt.float32
ALU = mybir.AluOpType
AF = mybir.ActivationFunctionType

# columns in the broadcast-constant tile
K1 = 0            # conv1_w taps  [k00,k01,k10,k11]
K2 = 4            # conv2_w taps
W1C = 8           # W1[j,i] -> 8 + 4j + i
B1C = 20          # b1[j]
W2C = 23          # W2[j,i] -> 23 + 3j + i
B2C = 29          # b2[j]
SH1 = 31          # -0.2 (conv1 relu shift)
SH2 = 32          # -2.0 (f relu shift)
NW = 33


def _build(repeat=1):
    """repeat>1 unrolls the whole pipeline R times inside one NEFF —
    used only by bench.py to measure steady-state per-pass time with a
    strong signal (device busy R*~100us >> dispatch noise)."""
    nc = bass.Bass("TRN2", target_bir_lowering=False, debug=False,
                   num_devices=N_CORES)
    x = nc.dram_tensor("x", [BC, 64], F32, kind="ExternalInput")
    wconst = nc.dram_tensor("wconst", [P, NW], F32, kind="ExternalInput")
    out = nc.dram_tensor("out", [BC, 4], F32, kind="ExternalOutput")

    CMAX = max(TILE_CS)
    GMAX = max(sum(TILE_CS[i] for i in g) for g in GROUPS)

    with tile.TileContext(nc) as tc:
        with (
            tc.tile_pool(name="consts", bufs=1) as cpool,
            tc.tile_pool(name="x", bufs=2) as xpool,
            tc.tile_pool(name="mid", bufs=2) as mpool,
            tc.tile_pool(name="small", bufs=2) as spool,
            tc.tile_pool(name="grp", bufs=2) as gpool,
            tc.tile_pool(name="out", bufs=2) as opool,
        ):
            ws = cpool.tile([P, NW], F32)
            nc.sync.dma_start(ws[:], wconst.ap()[:])

            def sc(col):
                return ws[:, col:col + 1]

            # ACT table warmup during the first x load: touch every
            # activation set used below so the ~2.7us PSEUDO_LOAD fires
            # at t~1us instead of ahead of the first compute tile.
            scratch = cpool.tile([P, 4], F32)
            nc.scalar.activation(scratch[:, 0:1], ws[:, SH1:SH1 + 1],
                                 AF.Relu, bias=0.0, scale=1.0)
            nc.scalar.activation(scratch[:, 1:2], ws[:, SH1:SH1 + 1],
                                 AF.Identity, bias=0.0, scale=1.0)
            nc.scalar.activation(scratch[:, 2:3], ws[:, SH1:SH1 + 1],
                                 AF.Sigmoid, bias=0.0, scale=1.0)

            def conv_tile(C, row0, f, off):
                """Load + conv1 + relu + conv2 + shifted-relu; writes the
                f_i features into f[:, off*4:(off+C)*4]."""
                x_view = x.ap()[row0:row0 + P * C, :].rearrange(
                    "(p c) f -> p (c f)", p=P, c=C)
                xt = xpool.tile([P, CMAX * 64], F32, tag="xt")
                nc.sync.dma_start(xt[:, :C * 64], x_view)

                # conv1: t1[c,oh,ow] = sum_t k[ti,tj]*x[c,2oh+ti,2ow+tj]
                xv = xt[:, :C * 64].rearrange(
                    "p (c oh ti ow tj) -> p c oh ti ow tj", oh=4, ti=2,
                    ow=4, tj=2)
                t1 = mpool.tile([P, CMAX * 16], F32, tag="t1")
                t1v = t1[:, :C * 16].rearrange("p (c oh ow) -> p c oh ow",
                                               oh=4, ow=4)
                nc.scalar.activation(t1v, xv[:, :, :, 0, :, 0], AF.Copy,
                                     bias=0.0, scale=sc(K1 + 0))
                nc.vector.scalar_tensor_tensor(
                    t1v, xv[:, :, :, 0, :, 1], sc(K1 + 1), t1v,
                    ALU.mult, ALU.add)
                nc.vector.scalar_tensor_tensor(
                    t1v, xv[:, :, :, 1, :, 0], sc(K1 + 2), t1v,
                    ALU.mult, ALU.add)
                nc.vector.scalar_tensor_tensor(
                    t1v, xv[:, :, :, 1, :, 1], sc(K1 + 3), t1v,
                    ALU.mult, ALU.add)

                # relu(x1 - 0.2)
                x1r = mpool.tile([P, CMAX * 16], F32, tag="x1r")
                nc.scalar.activation(x1r[:, :C * 16], t1[:, :C * 16],
                                     AF.Relu, bias=sc(SH1), scale=1.0)

                # conv2 on the 4x4 maps
                x1v = x1r[:, :C * 16].rearrange(
                    "p (c oh ti ow tj) -> p c oh ti ow tj", oh=2, ti=2,
                    ow=2, tj=2)
                t2 = spool.tile([P, CMAX * 4], F32, tag="t2")
                t2v = t2[:, :C * 4].rearrange("p (c oh ow) -> p c oh ow",
                                              oh=2, ow=2)
                # conv2 accumulate stays on DVE: GPSIMD elementwise was
                # measured 3x+ slower with SBUF port contention (a
                # ts_mul+tt_add variant regressed per-pass 70us -> 240us),
                # and walrus cannot lower STT on Pool at all.
                nc.scalar.activation(t2v, x1v[:, :, :, 0, :, 0], AF.Copy,
                                     bias=0.0, scale=sc(K2 + 0))
                nc.vector.scalar_tensor_tensor(
                    t2v, x1v[:, :, :, 0, :, 1], sc(K2 + 1), t2v,
                    ALU.mult, ALU.add)
                nc.vector.scalar_tensor_tensor(
                    t2v, x1v[:, :, :, 1, :, 0], sc(K2 + 2), t2v,
                    ALU.mult, ALU.add)
                nc.vector.scalar_tensor_tensor(
                    t2v, x1v[:, :, :, 1, :, 1], sc(K2 + 3), t2v,
                    ALU.mult, ALU.add)

                # f = relu(x2 - 2)
                nc.scalar.activation(f[:, off * 4:(off + C) * 4],
                                     t2[:, :C * 4], AF.Relu,
                                     bias=sc(SH2), scale=1.0)

            def mlp_group(f, G, chunks):
                """4->3 relu MLP, 3->2 linear, softmax(2), store."""
                fv = f[:, :G * 4].rearrange("p (c i) -> p c i", i=4)
                # h_j = relu(sum_i W1[j,i] f_i + b1_j), stored j-major
                h = gpool.tile([P, GMAX * 3], F32, tag="h")
                for j in range(3):
                    hj = h[:, j * G:(j + 1) * G]
                    nc.scalar.activation(hj, fv[:, :, 0], AF.Identity,
                                         bias=sc(B1C + j),
                                         scale=sc(W1C + 4 * j))
                    for i in range(1, 4):
                        nc.vector.scalar_tensor_tensor(
                            hj, fv[:, :, i], sc(W1C + 4 * j + i), hj,
                            ALU.mult, ALU.add)
                hr = gpool.tile([P, GMAX * 3], F32, tag="hr")
                nc.scalar.activation(hr[:, :G * 3], h[:, :G * 3], AF.Relu,
                                     bias=0.0, scale=1.0)
                hrv = hr[:, :G * 3].rearrange("p (j c) -> p j c", j=3)

                # out layout per sample: [cls0, cls1, raw0, raw1]
                ot = opool.tile([P, GMAX * 4], F32, tag="ot")
                ov = ot[:, :G * 4].rearrange("p (c four) -> p c four", four=4)
                for j in range(2):
                    rj = ov[:, :, 2 + j]
                    nc.scalar.activation(rj, hrv[:, 0, :], AF.Identity,
                                         bias=sc(B2C + j),
                                         scale=sc(W2C + 3 * j))
                    for i in range(1, 3):
                        nc.vector.scalar_tensor_tensor(
                            rj, hrv[:, i, :], sc(W2C + 3 * j + i), rj,
                            ALU.mult, ALU.add)

                # softmax over 2 classes: cls0 = sigmoid(raw0-raw1)
                d = gpool.tile([P, GMAX], F32, tag="d")
                nc.vector.tensor_sub(d[:, :G], ov[:, :, 2], ov[:, :, 3])
                nc.scalar.activation(ov[:, :, 0], d[:, :G], AF.Sigmoid,
                                     bias=0.0, scale=1.0)
                nc.scalar.activation(ov[:, :, 1], d[:, :G], AF.Sigmoid,
                                     bias=0.0, scale=-1.0)

                # Store per conv-tile chunk (sample rows are chunk-major)
                # on the SP ring: at ~716 GB/s/core the load chain has
                # plenty of slack, and keeping stores off the ACT queue
                # frees the ACT engine (a near-co-bottleneck).
                for off_c, C, row0 in chunks:
                    out_view = out.ap()[row0:row0 + P * C, :].rearrange(
                        "(p c) four -> p (c four)", p=P, c=C)
                    nc.sync.dma_start(out_view,
                                      ot[:, off_c * 4:(off_c + C) * 4])

            # Software-pipelined emission: group g's conv ops are issued
            # before group g-1's MLP so the next tiles' conv leads never
            # queue behind a full MLP chain on the ACT engine.
            for _rep in range(repeat):
                s0 = 0
                pending = None          # (f, G, chunks) awaiting MLP
                for group in GROUPS:
                    G = sum(TILE_CS[i] for i in group)
                    f = gpool.tile([P, GMAX * 4], F32, tag="f")
                    chunks = []
                    off = 0
                    for idx in group:
                        C = TILE_CS[idx]
                        conv_tile(C, s0, f, off)
                        chunks.append((off, C, s0))
                        off += C
                        s0 += P * C
                    if pending is not None:
                        mlp_group(*pending)
                    pending = (f, G, chunks)
                mlp_group(*pending)

    _split_multiwaits(nc)
    return nc


_NC = {}
LAST_RESULT = None  # BassKernelResults of the most recent kernel() call


def _get_nc(repeat=1):
    if repeat not in _NC:
        _NC[repeat] = _build(repeat)
    return _NC[repeat]


def prep_in_maps(x, conv1_w, conv2_w, W1, b1, W2, b2):
    x = np.ascontiguousarray(np.asarray(x, dtype=np.float32)).reshape(B, 64)
    row = np.concatenate([
        np.asarray(conv1_w, dtype=np.float32).reshape(4),
        np.asarray(conv2_w, dtype=np.float32).reshape(4),
        np.asarray(W1, dtype=np.float32).reshape(12),
        np.asarray(b1, dtype=np.float32).reshape(3),
        np.asarray(W2, dtype=np.float32).reshape(6),
        np.asarray(b2, dtype=np.float32).reshape(2),
        np.array([-0.2, -2.0], dtype=np.float32),
    ])
    wconst = np.ascontiguousarray(np.tile(row[None, :], (P, 1)))
    return [
        {"x": np.ascontiguousarray(x[i * BC:(i + 1) * BC]), "wconst": wconst}
        for i in range(N_CORES)
    ]


def kernel(x, conv1_w, conv2_w, W1, b1, W2, b2):
    in_maps = prep_in_maps(x, conv1_w, conv2_w, W1, b1, W2, b2)
    nc = _get_nc()
    res = run_bass_kernel_spmd(nc, in_maps, core_ids=list(range(N_CORES)))
    global LAST_RESULT
    LAST_RESULT = res
    out = np.concatenate([res.results[i]["out"] for i in range(N_CORES)], axis=0)
    classification = np.ascontiguousarray(out[:, 0:2])
    raw = np.ascontiguousarray(out[:, 2:4])
    return classification, raw

